# revision 16
# baseline (speedup 1.0000x reference)
"""Self-contained Trainium2 Bass kernel for nn_Attention (8-head self-attention).

Reference computation (per batch element b):
    xt = x[b].reshape(C, N).T            # (N, C),  N = H*W = 1024
    q  = xt @ Wq                         # (N, 512)
    k, v = split(xt @ Wkv)               # (N, 512) each
    per head h (d=64): sim = q_h k_h^T / 8 ; P = softmax(sim) ; o_h = P v_h
    out[b] = concat_h(o_h) @ Wo + bo     # (N, C)

Sharding: pure data parallel -- core b computes batch element b (8 cores, 8
batch elements, no collectives).

The PE stream is the wall (~378 matmuls x 512 free columns); ScalarE's exp
stream (64 x [128,1024], ~73us) fits underneath it.  Structure:
  - Inputs cast to bf16 on the HOST (half the DMA bytes, no on-chip casts).
    Critical inputs (x / Wk / Wq) interleave per 128-row chunk on the sync
    DMA queue; the mt0 k/q projection accumulates a-outer, chasing them.
  - Head pairs (2p, 2p+1) at partition halves of qT/kT tile p.  Per (pair,
    key-tile) beat: 4 sim MMs -> one [128,1024] exp per head (PSUM->SBUF
    bf16) -> 4 attn@v MMs three beats later.  attn@v keeps a per-head ones
    column (M=65) so softmax denominators cost no extra PE streaming.
  - v-projection tiles ride the psO pool inside pair 0 before the attn@v
    accumulators claim it; kq mt1-3 run as separate k/q beats inside pairs
    (never holding both psS bufs, so the sim/exp pipeline keeps flowing).
  - Norm chain per head: one [65,N] PSUM->SBUF copy (frees PSUM), DVE
    reciprocal_approx_fast on the [1,N] denominator row, DRAM-bounce
    broadcast to [64,N], one DVE multiply.  Deferred one pair to hide
    latency; only the last pair drains serially.
  - Output projection: kk=0..2 partials (+bias) run inside pair 3; the tail
    after the last exp only runs the kk=3 matmuls + final adds + stores.
  - exp LUT table-load (~2.7us) hides behind a warmup exp at t=0.

PSUM (8 banks): tag "st" 2x[128,1024]f32 (sim + projection beats) and tag
"ov" 2x[65->128,1024]f32 (attn@v accumulators / v-proj tiles).
"""

import numpy as np

import concourse.bass as bass
import concourse.mybir as mybir
import concourse.tile as tile
from concourse import bacc

B, C, N = 8, 512, 1024
HEADS, D = 8, 64
INNER = HEADS * D  # 512
SCALE = D ** -0.5
P = 128
CT = C // P       # 4  k-tiles over C
MT = INNER // P   # 4  partition-tiles over inner (one per head pair)
JT = N // P       # 8  key tiles
NT = N // P       # 8  output row tiles
NB = N // 512     # 2  free-dim blocks of 512 over N

F32 = mybir.dt.float32
BF16 = mybir.dt.bfloat16
EXP = mybir.ActivationFunctionType.Exp


def build_nc(debug=False):
    nc = bacc.Bacc(
        "TRN2", target_bir_lowering=False, debug=debug, num_devices=B
    )
    x_d = nc.dram_tensor("x", [C, N], BF16, kind="ExternalInput")
    wq_d = nc.dram_tensor("Wq", [C, INNER], BF16, kind="ExternalInput")
    wk_d = nc.dram_tensor("Wk", [C, INNER], BF16, kind="ExternalInput")
    wv_d = nc.dram_tensor("Wv", [C, INNER], BF16, kind="ExternalInput")
    wo_d = nc.dram_tensor("Wo", [INNER, C], BF16, kind="ExternalInput")
    bo_d = nc.dram_tensor("bo", [C], F32, kind="ExternalInput")
    out_d = nc.dram_tensor("out", [N, C], BF16, kind="ExternalOutput")

    with tile.TileContext(nc) as tc:
        with (
            tc.tile_pool(name="persist", bufs=1) as persist,
            tc.tile_pool(name="etp", bufs=4) as etp,
            tc.tile_pool(name="ovp", bufs=4) as ovp,
            tc.tile_pool(name="small", bufs=3) as small,
            tc.tile_pool(name="dramp", bufs=2, space="DRAM") as dramp,
            tc.tile_pool(name="psS", bufs=2, space="PSUM") as psS,
            tc.tile_pool(name="psO", bufs=2, space="PSUM") as psO,
        ):
            # ---------- consts + exp table warmup (ACT idle at t=0) ----------
            zb = persist.tile([P, 1], F32)
            nc.vector.memset(zb, 0.0)
            warm_i = persist.tile([1, 8], F32)
            nc.vector.memset(warm_i, 0.0)
            warm_o = persist.tile([1, 8], BF16)
            nc.scalar.activation(
                out=warm_o, in_=warm_i, func=EXP, bias=zb[0:1, :], scale=1.0)

            # ---------- input DMAs ----------
            # critical path (x, Wk, Wq chunks) interleaved on the sync queue;
            # Wv / Wo follow; bo broadcast on the gpsimd queue.
            x_b = persist.tile([P, CT, N], BF16)
            wk_b = persist.tile([P, CT, INNER], BF16)
            wq_b = persist.tile([P, CT, INNER], BF16)
            wv_b = persist.tile([P, CT, INNER], BF16)
            x_dv = x_d[:].rearrange("(a p) n -> p a n", p=P)
            wk_dv = wk_d[:].rearrange("(a p) m -> p a m", p=P)
            wq_dv = wq_d[:].rearrange("(a p) m -> p a m", p=P)
            wv_dv = wv_d[:].rearrange("(a p) m -> p a m", p=P)
            # x chunks chase on the sync queue; Wk / Wq / Wv each ride an
            # otherwise-idle engine's queue so DMA issue overhead overlaps
            for a in range(CT):
                nc.sync.dma_start(out=x_b[:, a, :], in_=x_dv[:, a, :])
            nc.scalar.dma_start(out=wk_b, in_=wk_dv)
            nc.scalar.dma_start(out=wq_b, in_=wq_dv)
            nc.gpsimd.dma_start(out=wv_b, in_=wv_dv)
            wo_b = persist.tile([P, MT, C], BF16)
            nc.gpsimd.dma_start(
                out=wo_b, in_=wo_d[:].rearrange("(a p) m -> p a m", p=P))
            bo_bc = persist.tile([P, C], F32)
            bo_ap = bo_d[:]
            nc.gpsimd.dma_start(
                out=bo_bc,
                in_=bass.AP(tensor=bo_ap.tensor, offset=bo_ap.offset,
                            ap=[[0, P], [1, C]]),
            )

            # ---------- persistent attention tensors ----------
            qT = persist.tile([P, MT, N], BF16)
            kT = persist.tile([P, MT, N], BF16)
            v_ext = persist.tile([P, JT, HEADS, D + 1], BF16)
            nc.vector.memset(v_ext[:, :, :, D], 1.0)
            oTs = []
            for m in range(MT):
                oT_m = persist.tile([P, N], BF16, tag=f"oT{m}")
                oTs.append(oT_m)
            part_sb = persist.tile([P, NT, C], F32)

            def proj_beat(mt, w_b, dst):
                """k or q projection for inner tile mt: one psS buf held over
                the a-contraction, then one DVE copy out."""
                acc = psS.tile([P, N], F32, tag="st")
                for a in range(CT):
                    for ib in range(NB):
                        nc.tensor.matmul(
                            acc[:, ib * 512:(ib + 1) * 512],
                            lhsT=w_b[:, a, mt * P:(mt + 1) * P],
                            rhs=x_b[:, a, ib * 512:(ib + 1) * 512],
                            start=(a == 0),
                            stop=(a == CT - 1),
                        )
                nc.vector.tensor_copy(out=dst[:, mt, :], in_=acc)

            def v_beat(jt):
                """V for all heads of key tile jt, through the psO pool."""
                ps = psO.tile([P, INNER], F32, tag="ov")
                for a in range(CT):
                    nc.tensor.matmul(
                        ps,
                        lhsT=x_b[:, a, jt * P:(jt + 1) * P],
                        rhs=wv_b[:, a, :],
                        start=(a == 0),
                        stop=(a == CT - 1),
                    )
                nc.vector.tensor_copy(
                    out=v_ext[:, jt, :, 0:D],
                    in_=ps.rearrange("p (h d) -> p h d", h=HEADS),
                )

            def out_partial(it):
                """Output projection kk=0..2 partial for row tile it, plus
                bias, parked in SBUF; the tail only needs the kk=3 matmul."""
                pp = psS.tile([P, C], F32, tag="st")
                for kk in range(MT - 1):
                    nc.tensor.matmul(
                        pp,
                        lhsT=oTs[kk][:, it * P:(it + 1) * P],
                        rhs=wo_b[:, kk, :],
                        start=(kk == 0),
                        stop=(kk == MT - 2),
                    )
                nc.vector.tensor_add(part_sb[:, it, :], pp, bo_bc)

            # ---------- attention ----------
            def sim_beat(p, jt, etA, etB):
                stA = psS.tile([P, N], F32, tag="st")
                stB = psS.tile([P, N], F32, tag="st")
                for ib in range(NB):
                    for st, base in ((stA, 0), (stB, D)):
                        nc.tensor.matmul(
                            st[:, ib * 512:(ib + 1) * 512],
                            lhsT=kT[base:base + D, p, jt * P:(jt + 1) * P],
                            rhs=qT[base:base + D, p, ib * 512:(ib + 1) * 512],
                            start=True,
                            stop=True,
                        )
                nc.scalar.activation(
                    out=etA[:, jt, :], in_=stA, func=EXP, bias=zb, scale=SCALE)
                nc.scalar.activation(
                    out=etB[:, jt, :], in_=stB, func=EXP, bias=zb, scale=SCALE)

            def av_beat(p, jt, etA, etB, ovA, ovB):
                for ov, et, h in ((ovA, etA, 2 * p), (ovB, etB, 2 * p + 1)):
                    for ib in range(NB):
                        nc.tensor.matmul(
                            ov[:, ib * 512:(ib + 1) * 512],
                            lhsT=v_ext[:, jt, h, :],
                            rhs=et[:, jt, ib * 512:(ib + 1) * 512],
                            start=(jt == 0),
                            stop=(jt == JT - 1),
                        )

            class Pend:
                pass

            def avtail_and_recip(pend):
                """av jt=7 for the previous pair + PSUM release; denominator
                row spread across 128 partitions via DRAM so the (slow
                per-element) DVE reciprocal runs on 8 elements per lane."""
                av_beat(pend.p, JT - 1, pend.etA, pend.etB, pend.ovA, pend.ovB)
                pend.sb = []
                pend.sds = []
                for ov in (pend.ovA, pend.ovB):
                    ov_sb = ovp.tile([D + 1, N], F32, tag="ovsb")
                    nc.vector.tensor_copy(out=ov_sb, in_=ov)  # frees psO buf
                    sd = dramp.tile([N], F32, tag="sd")
                    nc.sync.dma_start(out=sd, in_=ov_sb[D:D + 1, :])
                    pend.sb.append(ov_sb)
                    pend.sds.append(sd)

            def rep_dma(pend):
                pend.rep = []
                for sd in pend.sds:
                    st2 = small.tile([P, NT], F32, tag="st2")
                    nc.sync.dma_start(
                        out=st2, in_=sd.rearrange("(p k) -> p k", k=NT))
                    rst2 = small.tile([P, NT], F32, tag="rst2")
                    nc.vector.reciprocal(rst2, st2)
                    rsd = dramp.tile([N], F32, tag="rsd")
                    nc.sync.dma_start(
                        out=rsd.rearrange("(p k) -> p k", k=NT), in_=rst2)
                    rep = small.tile([D, N], F32, tag="rep")
                    rsd_ap = rsd[:]
                    nc.sync.dma_start(
                        out=rep,
                        in_=bass.AP(tensor=rsd_ap.tensor, offset=rsd_ap.offset,
                                    ap=[[0, D], [1, N]]),
                    )
                    pend.rep.append(rep)

            def norm_mul(pend):
                for i, base in ((0, 0), (1, D)):
                    nc.vector.tensor_mul(
                        oTs[pend.p][base:base + D, :],
                        pend.sb[i][0:D, :], pend.rep[i])

            # ---------- emission ----------
            proj_beat(0, wk_b, kT)
            proj_beat(0, wq_b, qT)

            pend = None
            for p in range(MT):
                etA = etp.tile([P, JT, N], BF16, tag="et")
                etB = etp.tile([P, JT, N], BF16, tag="et")
                ovA = ovB = None
                for jt in range(JT):
                    sim_beat(p, jt, etA, etB)
                    if p == 0:
                        # v tiles ride psO before ovA/ovB claim it
                        if jt == 0:
                            v_beat(0), v_beat(1), v_beat(2)
                        elif jt == 1:
                            v_beat(3), v_beat(4)
                        elif jt == 2:
                            v_beat(5), v_beat(6), v_beat(7)
                    # last pair: compress the pending chain so oTs[2] exists
                    # before the output-projection partial beats
                    mul_jt = 4 if p == MT - 1 else 5
                    rep_jt = 2 if p == MT - 1 else 3
                    if pend is not None:
                        if jt == 1:
                            avtail_and_recip(pend)
                        elif jt == rep_jt:
                            rep_dma(pend)
                        elif jt == mul_jt:
                            norm_mul(pend)
                            pend = None
                    if jt == 3:
                        ovA = psO.tile([D + 1, N], F32, tag="ov")
                        ovB = psO.tile([D + 1, N], F32, tag="ov")
                    if jt >= 3:
                        av_beat(p, jt - 3, etA, etB, ovA, ovB)
                    # backfilled projection / output-partial beats
                    if p < MT - 1:
                        if jt == 4:
                            proj_beat(p + 1, wk_b, kT)
                        elif jt == 6:
                            proj_beat(p + 1, wq_b, qT)
                    else:
                        if jt >= 5:
                            base_it = 2 * (jt - 5)
                            out_partial(base_it)
                            out_partial(base_it + 1)
                av_beat(p, JT - 3, etA, etB, ovA, ovB)
                av_beat(p, JT - 2, etA, etB, ovA, ovB)
                newp = Pend()
                newp.p, newp.etA, newp.etB, newp.ovA, newp.ovB = \
                    p, etA, etB, ovA, ovB
                pend = newp

            # drain the last pair: norm-chain DVE ops go FIRST on the DVE
            # queue; the last two output partials keep the PE busy meanwhile
            avtail_and_recip(pend)
            out_partial(NT - 2)
            out_partial(NT - 1)
            rep_dma(pend)
            norm_mul(pend)

            # ---------- output projection tail: kk=3 + partial + store ------
            for it in range(NT):
                pf = psS.tile([P, C], F32, tag="st")
                nc.tensor.matmul(
                    pf,
                    lhsT=oTs[MT - 1][:, it * P:(it + 1) * P],
                    rhs=wo_b[:, MT - 1, :],
                    start=True,
                    stop=True,
                )
                fin = small.tile([P, C], BF16, tag="fin")
                nc.vector.tensor_add(fin, pf, part_sb[:, it, :])
                eng = nc.sync if it % 2 == 0 else nc.gpsimd
                eng.dma_start(out=out_d[it * P:(it + 1) * P, :], in_=fin)

    return nc


BFNP = mybir.dt.np(BF16)


def prepare_in_maps(x, Wq, Wkv, Wo, bo):
    """Host-side prep: reshape x per core, split Wkv, cast matmul inputs
    to bf16 (they feed bf16 matmuls on-chip either way)."""
    x = np.ascontiguousarray(np.asarray(x, np.float32).reshape(B, C, N))
    wkv = np.asarray(Wkv, np.float32)
    wq = np.asarray(Wq, np.float32).astype(BFNP)
    wk = np.ascontiguousarray(wkv[:, :INNER]).astype(BFNP)
    wv = np.ascontiguousarray(wkv[:, INNER:]).astype(BFNP)
    wo = np.asarray(Wo, np.float32).astype(BFNP)
    bo = np.asarray(bo, np.float32)
    return [
        {"x": x[b].astype(BFNP), "Wq": wq, "Wk": wk, "Wv": wv, "Wo": wo,
         "bo": bo}
        for b in range(B)
    ]


def kernel(x, Wq, Wkv, Wo, bo):
    from concourse.bass_utils import run_bass_kernel_spmd

    nc = build_nc()
    nc.compile()
    in_maps = prepare_in_maps(x, Wq, Wkv, Wo, bo)
    res = run_bass_kernel_spmd(nc, in_maps, list(range(B)))
    return np.stack(
        [np.asarray(res.results[b]["out"], dtype=np.float32) for b in range(B)],
        axis=0)


# revision 17
# speedup vs baseline: 1.0280x; 1.0280x over previous
"""Self-contained Trainium2 Bass kernel for nn_Attention (8-head self-attention).

Reference computation (per batch element b):
    xt = x[b].reshape(C, N).T            # (N, C),  N = H*W = 1024
    q  = xt @ Wq                         # (N, 512)
    k, v = split(xt @ Wkv)               # (N, 512) each
    per head h (d=64): sim = q_h k_h^T / 8 ; P = softmax(sim) ; o_h = P v_h
    out[b] = concat_h(o_h) @ Wo + bo     # (N, C)

Sharding: pure data parallel -- core b computes batch element b (8 cores, 8
batch elements, no collectives).

The PE stream is the wall (~378 matmuls x 512 free columns); ScalarE's exp
stream (64 x [128,1024], ~73us) fits underneath it.  Structure:
  - Inputs cast to bf16 on the HOST (half the DMA bytes, no on-chip casts).
    Critical inputs (x / Wk / Wq) interleave per 128-row chunk on the sync
    DMA queue; the mt0 k/q projection accumulates a-outer, chasing them.
  - Head pairs (2p, 2p+1) at partition halves of qT/kT tile p.  Per (pair,
    key-tile) beat: 4 sim MMs -> one [128,1024] exp per head (PSUM->SBUF
    bf16) -> 4 attn@v MMs three beats later.  attn@v keeps a per-head ones
    column (M=65) so softmax denominators cost no extra PE streaming.
  - v-projection tiles ride the psO pool inside pair 0 before the attn@v
    accumulators claim it; kq mt1-3 run as separate k/q beats inside pairs
    (never holding both psS bufs, so the sim/exp pipeline keeps flowing).
  - Norm chain per head: one [65,N] PSUM->SBUF copy (frees PSUM), DVE
    reciprocal_approx_fast on the [1,N] denominator row, DRAM-bounce
    broadcast to [64,N], one DVE multiply.  Deferred one pair to hide
    latency; only the last pair drains serially.
  - Output projection: kk=0..2 partials (+bias) run inside pair 3; the tail
    after the last exp only runs the kk=3 matmuls + final adds + stores.
  - exp LUT table-load (~2.7us) hides behind a warmup exp at t=0.

PSUM (8 banks): tag "st" 2x[128,1024]f32 (sim + projection beats) and tag
"ov" 2x[65->128,1024]f32 (attn@v accumulators / v-proj tiles).
"""

import numpy as np

import concourse.bass as bass
import concourse.mybir as mybir
import concourse.tile as tile
from concourse import bacc

B, C, N = 8, 512, 1024
HEADS, D = 8, 64
INNER = HEADS * D  # 512
SCALE = D ** -0.5
P = 128
CT = C // P       # 4  k-tiles over C
MT = INNER // P   # 4  partition-tiles over inner (one per head pair)
JT = N // P       # 8  key tiles
NT = N // P       # 8  output row tiles
NB = N // 512     # 2  free-dim blocks of 512 over N

F32 = mybir.dt.float32
BF16 = mybir.dt.bfloat16
EXP = mybir.ActivationFunctionType.Exp


def build_nc(debug=False):
    nc = bacc.Bacc(
        "TRN2", target_bir_lowering=False, debug=debug, num_devices=B
    )
    x_d = nc.dram_tensor("x", [C, N], BF16, kind="ExternalInput")
    wq_d = nc.dram_tensor("Wq", [C, INNER], BF16, kind="ExternalInput")
    wk_d = nc.dram_tensor("Wk", [C, INNER], BF16, kind="ExternalInput")
    wv_d = nc.dram_tensor("Wv", [C, INNER], BF16, kind="ExternalInput")
    wo_d = nc.dram_tensor("Wo", [INNER, C], BF16, kind="ExternalInput")
    bo_d = nc.dram_tensor("bo", [C], F32, kind="ExternalInput")
    out_d = nc.dram_tensor("out", [N, C], BF16, kind="ExternalOutput")

    with tile.TileContext(nc) as tc:
        with (
            tc.tile_pool(name="persist", bufs=1) as persist,
            tc.tile_pool(name="etp", bufs=4) as etp,
            tc.tile_pool(name="ovp", bufs=4) as ovp,
            tc.tile_pool(name="small", bufs=3) as small,
            tc.tile_pool(name="dramp", bufs=2, space="DRAM") as dramp,
            tc.tile_pool(name="psS", bufs=2, space="PSUM") as psS,
            tc.tile_pool(name="psO", bufs=2, space="PSUM") as psO,
        ):
            # ---------- consts + exp table warmup (ACT idle at t=0) ----------
            zb = persist.tile([P, 1], F32)
            nc.vector.memset(zb, 0.0)
            warm_i = persist.tile([1, 8], F32)
            nc.vector.memset(warm_i, 0.0)
            warm_o = persist.tile([1, 8], BF16)
            nc.scalar.activation(
                out=warm_o, in_=warm_i, func=EXP, bias=zb[0:1, :], scale=1.0)

            # ---------- input DMAs ----------
            # critical path (x, Wk, Wq chunks) interleaved on the sync queue;
            # Wv / Wo follow; bo broadcast on the gpsimd queue.
            x_b = persist.tile([P, CT, N], BF16)
            wk_b = persist.tile([P, CT, INNER], BF16)
            wq_b = persist.tile([P, CT, INNER], BF16)
            wv_b = persist.tile([P, CT, INNER], BF16)
            x_dv = x_d[:].rearrange("(a p) n -> p a n", p=P)
            wk_dv = wk_d[:].rearrange("(a p) m -> p a m", p=P)
            wq_dv = wq_d[:].rearrange("(a p) m -> p a m", p=P)
            wv_dv = wv_d[:].rearrange("(a p) m -> p a m", p=P)
            for a in range(CT):
                nc.sync.dma_start(out=x_b[:, a, :], in_=x_dv[:, a, :])
                nc.sync.dma_start(out=wk_b[:, a, :], in_=wk_dv[:, a, :])
                nc.sync.dma_start(out=wq_b[:, a, :], in_=wq_dv[:, a, :])
            for a in range(CT):
                nc.sync.dma_start(out=wv_b[:, a, :], in_=wv_dv[:, a, :])
            wo_b = persist.tile([P, MT, C], BF16)
            nc.sync.dma_start(
                out=wo_b, in_=wo_d[:].rearrange("(a p) m -> p a m", p=P))
            bo_bc = persist.tile([P, C], F32)
            bo_ap = bo_d[:]
            nc.gpsimd.dma_start(
                out=bo_bc,
                in_=bass.AP(tensor=bo_ap.tensor, offset=bo_ap.offset,
                            ap=[[0, P], [1, C]]),
            )

            # ---------- persistent attention tensors ----------
            qT = persist.tile([P, MT, N], BF16)
            kT = persist.tile([P, MT, N], BF16)
            v_ext = persist.tile([P, JT, HEADS, D + 1], BF16)
            nc.vector.memset(v_ext[:, :, :, D], 1.0)
            oTs = []
            for m in range(MT):
                oT_m = persist.tile([P, N], BF16, tag=f"oT{m}")
                oTs.append(oT_m)
            part_sb = persist.tile([P, NT, C], F32)

            def proj_beat(mt, w_b, dst):
                """k or q projection for inner tile mt: one psS buf held over
                the a-contraction, then one DVE copy out."""
                acc = psS.tile([P, N], F32, tag="st")
                for a in range(CT):
                    for ib in range(NB):
                        nc.tensor.matmul(
                            acc[:, ib * 512:(ib + 1) * 512],
                            lhsT=w_b[:, a, mt * P:(mt + 1) * P],
                            rhs=x_b[:, a, ib * 512:(ib + 1) * 512],
                            start=(a == 0),
                            stop=(a == CT - 1),
                        )
                nc.vector.tensor_copy(out=dst[:, mt, :], in_=acc)

            def v_beat(jt):
                """V for all heads of key tile jt, through the psO pool."""
                ps = psO.tile([P, INNER], F32, tag="ov")
                for a in range(CT):
                    nc.tensor.matmul(
                        ps,
                        lhsT=x_b[:, a, jt * P:(jt + 1) * P],
                        rhs=wv_b[:, a, :],
                        start=(a == 0),
                        stop=(a == CT - 1),
                    )
                nc.vector.tensor_copy(
                    out=v_ext[:, jt, :, 0:D],
                    in_=ps.rearrange("p (h d) -> p h d", h=HEADS),
                )

            def out_partial(it):
                """Output projection kk=0..2 partial for row tile it, plus
                bias, parked in SBUF; the tail only needs the kk=3 matmul."""
                pp = psS.tile([P, C], F32, tag="st")
                for kk in range(MT - 1):
                    nc.tensor.matmul(
                        pp,
                        lhsT=oTs[kk][:, it * P:(it + 1) * P],
                        rhs=wo_b[:, kk, :],
                        start=(kk == 0),
                        stop=(kk == MT - 2),
                    )
                nc.vector.tensor_add(part_sb[:, it, :], pp, bo_bc)

            # ---------- attention ----------
            def sim_beat(p, jt, etA, etB):
                stA = psS.tile([P, N], F32, tag="st")
                stB = psS.tile([P, N], F32, tag="st")
                for ib in range(NB):
                    for st, base in ((stA, 0), (stB, D)):
                        nc.tensor.matmul(
                            st[:, ib * 512:(ib + 1) * 512],
                            lhsT=kT[base:base + D, p, jt * P:(jt + 1) * P],
                            rhs=qT[base:base + D, p, ib * 512:(ib + 1) * 512],
                            start=True,
                            stop=True,
                        )
                nc.scalar.activation(
                    out=etA[:, jt, :], in_=stA, func=EXP, bias=zb, scale=SCALE)
                nc.scalar.activation(
                    out=etB[:, jt, :], in_=stB, func=EXP, bias=zb, scale=SCALE)

            def av_beat(p, jt, etA, etB, ovA, ovB):
                for ov, et, h in ((ovA, etA, 2 * p), (ovB, etB, 2 * p + 1)):
                    for ib in range(NB):
                        nc.tensor.matmul(
                            ov[:, ib * 512:(ib + 1) * 512],
                            lhsT=v_ext[:, jt, h, :],
                            rhs=et[:, jt, ib * 512:(ib + 1) * 512],
                            start=(jt == 0),
                            stop=(jt == JT - 1),
                        )

            class Pend:
                pass

            def avtail_and_recip(pend):
                """av jt=7 for the previous pair + PSUM release; denominator
                row spread across 128 partitions via DRAM so the (slow
                per-element) DVE reciprocal runs on 8 elements per lane."""
                av_beat(pend.p, JT - 1, pend.etA, pend.etB, pend.ovA, pend.ovB)
                pend.sb = []
                pend.sds = []
                for ov in (pend.ovA, pend.ovB):
                    ov_sb = ovp.tile([D + 1, N], F32, tag="ovsb")
                    nc.vector.tensor_copy(out=ov_sb, in_=ov)  # frees psO buf
                    sd = dramp.tile([N], F32, tag="sd")
                    nc.sync.dma_start(out=sd, in_=ov_sb[D:D + 1, :])
                    pend.sb.append(ov_sb)
                    pend.sds.append(sd)

            def rep_dma(pend):
                pend.rep = []
                for sd in pend.sds:
                    st2 = small.tile([P, NT], F32, tag="st2")
                    nc.sync.dma_start(
                        out=st2, in_=sd.rearrange("(p k) -> p k", k=NT))
                    rst2 = small.tile([P, NT], F32, tag="rst2")
                    nc.vector.reciprocal(rst2, st2)
                    rsd = dramp.tile([N], F32, tag="rsd")
                    nc.sync.dma_start(
                        out=rsd.rearrange("(p k) -> p k", k=NT), in_=rst2)
                    rep = small.tile([D, N], F32, tag="rep")
                    rsd_ap = rsd[:]
                    nc.sync.dma_start(
                        out=rep,
                        in_=bass.AP(tensor=rsd_ap.tensor, offset=rsd_ap.offset,
                                    ap=[[0, D], [1, N]]),
                    )
                    pend.rep.append(rep)

            def norm_mul(pend):
                for i, base in ((0, 0), (1, D)):
                    nc.vector.tensor_mul(
                        oTs[pend.p][base:base + D, :],
                        pend.sb[i][0:D, :], pend.rep[i])

            # ---------- emission ----------
            proj_beat(0, wk_b, kT)
            proj_beat(0, wq_b, qT)

            pend = None
            for p in range(MT):
                etA = etp.tile([P, JT, N], BF16, tag="et")
                etB = etp.tile([P, JT, N], BF16, tag="et")
                ovA = ovB = None
                for jt in range(JT):
                    sim_beat(p, jt, etA, etB)
                    if p == 0:
                        # v tiles ride psO before ovA/ovB claim it
                        if jt == 0:
                            v_beat(0), v_beat(1), v_beat(2)
                        elif jt == 1:
                            v_beat(3), v_beat(4)
                        elif jt == 2:
                            v_beat(5), v_beat(6), v_beat(7)
                    # last pair: compress the pending chain so oTs[2] exists
                    # before the output-projection partial beats
                    mul_jt = 4 if p == MT - 1 else 5
                    rep_jt = 2 if p == MT - 1 else 3
                    if pend is not None:
                        if jt == 1:
                            avtail_and_recip(pend)
                        elif jt == rep_jt:
                            rep_dma(pend)
                        elif jt == mul_jt:
                            norm_mul(pend)
                            pend = None
                    if jt == 3:
                        ovA = psO.tile([D + 1, N], F32, tag="ov")
                        ovB = psO.tile([D + 1, N], F32, tag="ov")
                    if jt >= 3:
                        av_beat(p, jt - 3, etA, etB, ovA, ovB)
                    # backfilled projection / output-partial beats
                    if p < MT - 1:
                        if jt == 4:
                            proj_beat(p + 1, wk_b, kT)
                        elif jt == 6:
                            proj_beat(p + 1, wq_b, qT)
                    else:
                        if jt >= 5:
                            base_it = 2 * (jt - 5)
                            out_partial(base_it)
                            out_partial(base_it + 1)
                av_beat(p, JT - 3, etA, etB, ovA, ovB)
                av_beat(p, JT - 2, etA, etB, ovA, ovB)
                newp = Pend()
                newp.p, newp.etA, newp.etB, newp.ovA, newp.ovB = \
                    p, etA, etB, ovA, ovB
                pend = newp

            # drain the last pair: norm-chain DVE ops go FIRST on the DVE
            # queue; the last two output partials keep the PE busy meanwhile
            avtail_and_recip(pend)
            out_partial(NT - 2)
            out_partial(NT - 1)
            rep_dma(pend)
            norm_mul(pend)

            # ---------- output projection tail: kk=3 + partial + store ------
            for it in range(NT):
                pf = psS.tile([P, C], F32, tag="st")
                nc.tensor.matmul(
                    pf,
                    lhsT=oTs[MT - 1][:, it * P:(it + 1) * P],
                    rhs=wo_b[:, MT - 1, :],
                    start=True,
                    stop=True,
                )
                fin = small.tile([P, C], BF16, tag="fin")
                nc.vector.tensor_add(fin, pf, part_sb[:, it, :])
                eng = nc.sync if it % 2 == 0 else nc.gpsimd
                eng.dma_start(out=out_d[it * P:(it + 1) * P, :], in_=fin)

    return nc


BFNP = mybir.dt.np(BF16)


def prepare_in_maps(x, Wq, Wkv, Wo, bo):
    """Host-side prep: reshape x per core, split Wkv, cast matmul inputs
    to bf16 (they feed bf16 matmuls on-chip either way)."""
    x = np.ascontiguousarray(np.asarray(x, np.float32).reshape(B, C, N))
    wkv = np.asarray(Wkv, np.float32)
    wq = np.asarray(Wq, np.float32).astype(BFNP)
    wk = np.ascontiguousarray(wkv[:, :INNER]).astype(BFNP)
    wv = np.ascontiguousarray(wkv[:, INNER:]).astype(BFNP)
    wo = np.asarray(Wo, np.float32).astype(BFNP)
    bo = np.asarray(bo, np.float32)
    return [
        {"x": x[b].astype(BFNP), "Wq": wq, "Wk": wk, "Wv": wv, "Wo": wo,
         "bo": bo}
        for b in range(B)
    ]


def kernel(x, Wq, Wkv, Wo, bo):
    from concourse.bass_utils import run_bass_kernel_spmd

    nc = build_nc()
    nc.compile()
    in_maps = prepare_in_maps(x, Wq, Wkv, Wo, bo)
    res = run_bass_kernel_spmd(nc, in_maps, list(range(B)))
    return np.stack(
        [np.asarray(res.results[b]["out"], dtype=np.float32) for b in range(B)],
        axis=0)


# revision 18
# speedup vs baseline: 1.1689x; 1.1371x over previous
"""Self-contained Trainium2 Bass kernel for nn_Attention (8-head self-attention).

Reference computation (per batch element b):
    xt = x[b].reshape(C, N).T            # (N, C),  N = H*W = 1024
    q  = xt @ Wq                         # (N, 512)
    k, v = split(xt @ Wkv)               # (N, 512) each
    per head h (d=64): sim = q_h k_h^T / 8 ; P = softmax(sim) ; o_h = P v_h
    out[b] = concat_h(o_h) @ Wo + bo     # (N, C)

Sharding: pure data parallel -- core b computes batch element b (8 cores, 8
batch elements, no collectives).

The PE stream is the wall (~378 matmuls x 512 free columns); ScalarE's exp
stream (64 x [128,1024], ~73us) fits underneath it.  Structure:
  - Inputs cast to bf16 on the HOST (half the DMA bytes, no on-chip casts).
    Critical inputs (x / Wk / Wq) interleave per 128-row chunk on the sync
    DMA queue; the mt0 k/q projection accumulates a-outer, chasing them.
  - Head pairs (2p, 2p+1) at partition halves of qT/kT tile p.  Per (pair,
    key-tile) beat: 4 sim MMs -> one [128,1024] exp per head (PSUM->SBUF
    bf16) -> 4 attn@v MMs three beats later.  attn@v keeps a per-head ones
    column (M=65) so softmax denominators cost no extra PE streaming.
  - v-projection tiles ride the psO pool inside pair 0 before the attn@v
    accumulators claim it; kq mt1-3 run as separate k/q beats inside pairs
    (never holding both psS bufs, so the sim/exp pipeline keeps flowing).
  - Norm chain per head: one [65,N] PSUM->SBUF copy (frees PSUM), DVE
    reciprocal_approx_fast on the [1,N] denominator row, DRAM-bounce
    broadcast to [64,N], one DVE multiply.  Deferred one pair to hide
    latency; only the last pair drains serially.
  - Output projection: kk=0..2 partials (+bias) run inside pair 3; the tail
    after the last exp only runs the kk=3 matmuls + final adds + stores.
  - exp LUT table-load (~2.7us) hides behind a warmup exp at t=0.

PSUM (8 banks): tag "st" 2x[128,1024]f32 (sim + projection beats) and tag
"ov" 2x[65->128,1024]f32 (attn@v accumulators / v-proj tiles).
"""

import numpy as np

import concourse.bass as bass
import concourse.mybir as mybir
import concourse.tile as tile
from concourse import bacc

B, C, N = 8, 512, 1024
HEADS, D = 8, 64
INNER = HEADS * D  # 512
SCALE = D ** -0.5
P = 128
CT = C // P       # 4  k-tiles over C
MT = INNER // P   # 4  partition-tiles over inner (one per head pair)
JT = N // P       # 8  key tiles
NT = N // P       # 8  output row tiles
NB = N // 512     # 2  free-dim blocks of 512 over N

F32 = mybir.dt.float32
BF16 = mybir.dt.bfloat16
EXP = mybir.ActivationFunctionType.Exp


def build_nc(debug=False):
    nc = bacc.Bacc(
        "TRN2", target_bir_lowering=False, debug=debug, num_devices=B
    )
    x_d = nc.dram_tensor("x", [C, N], BF16, kind="ExternalInput")
    wq_d = nc.dram_tensor("Wq", [C, INNER], BF16, kind="ExternalInput")
    wk_d = nc.dram_tensor("Wk", [C, INNER], BF16, kind="ExternalInput")
    wv_d = nc.dram_tensor("Wv", [C, INNER], BF16, kind="ExternalInput")
    wo_d = nc.dram_tensor("Wo", [INNER, C], BF16, kind="ExternalInput")
    bo_d = nc.dram_tensor("bo", [C], F32, kind="ExternalInput")
    out_d = nc.dram_tensor("out", [N, C], F32, kind="ExternalOutput")

    with tile.TileContext(nc) as tc:
        with (
            tc.tile_pool(name="persist", bufs=1) as persist,
            tc.tile_pool(name="etp", bufs=4) as etp,
            tc.tile_pool(name="ovp", bufs=4) as ovp,
            tc.tile_pool(name="small", bufs=3) as small,
            tc.tile_pool(name="dramp", bufs=2, space="DRAM") as dramp,
            tc.tile_pool(name="psS", bufs=2, space="PSUM") as psS,
            tc.tile_pool(name="psO", bufs=2, space="PSUM") as psO,
        ):
            # ---------- consts + exp table warmup (ACT idle at t=0) ----------
            zb = persist.tile([P, 1], F32)
            nc.vector.memset(zb, 0.0)
            warm_i = persist.tile([1, 8], F32)
            nc.vector.memset(warm_i, 0.0)
            warm_o = persist.tile([1, 8], BF16)
            nc.scalar.activation(
                out=warm_o, in_=warm_i, func=EXP, bias=zb[0:1, :], scale=1.0)

            # ---------- input DMAs ----------
            # critical path (x, Wk, Wq chunks) interleaved on the sync queue;
            # Wv / Wo follow; bo broadcast on the gpsimd queue.
            x_b = persist.tile([P, CT, N], BF16)
            wk_b = persist.tile([P, CT, INNER], BF16)
            wq_b = persist.tile([P, CT, INNER], BF16)
            wv_b = persist.tile([P, CT, INNER], BF16)
            x_dv = x_d[:].rearrange("(a p) n -> p a n", p=P)
            wk_dv = wk_d[:].rearrange("(a p) m -> p a m", p=P)
            wq_dv = wq_d[:].rearrange("(a p) m -> p a m", p=P)
            wv_dv = wv_d[:].rearrange("(a p) m -> p a m", p=P)
            for a in range(CT):
                nc.sync.dma_start(out=x_b[:, a, :], in_=x_dv[:, a, :])
                nc.sync.dma_start(out=wk_b[:, a, :], in_=wk_dv[:, a, :])
                nc.sync.dma_start(out=wq_b[:, a, :], in_=wq_dv[:, a, :])
            for a in range(CT):
                nc.sync.dma_start(out=wv_b[:, a, :], in_=wv_dv[:, a, :])
            wo_b = persist.tile([P, MT, C], BF16)
            nc.sync.dma_start(
                out=wo_b, in_=wo_d[:].rearrange("(a p) m -> p a m", p=P))
            bo_bc = persist.tile([P, C], F32)
            bo_ap = bo_d[:]
            nc.gpsimd.dma_start(
                out=bo_bc,
                in_=bass.AP(tensor=bo_ap.tensor, offset=bo_ap.offset,
                            ap=[[0, P], [1, C]]),
            )

            # ---------- persistent attention tensors ----------
            qT = persist.tile([P, MT, N], BF16)
            kT = persist.tile([P, MT, N], BF16)
            v_ext = persist.tile([P, JT, HEADS, D + 1], BF16)
            nc.vector.memset(v_ext[:, :, :, D], 1.0)
            oTs = []
            for m in range(MT):
                oT_m = persist.tile([P, N], BF16, tag=f"oT{m}")
                oTs.append(oT_m)
            part_sb = persist.tile([P, NT, C], F32)

            def proj_beat(mt, w_b, dst):
                """k or q projection for inner tile mt: one psS buf held over
                the a-contraction, then one DVE copy out."""
                acc = psS.tile([P, N], F32, tag="st")
                for a in range(CT):
                    for ib in range(NB):
                        nc.tensor.matmul(
                            acc[:, ib * 512:(ib + 1) * 512],
                            lhsT=w_b[:, a, mt * P:(mt + 1) * P],
                            rhs=x_b[:, a, ib * 512:(ib + 1) * 512],
                            start=(a == 0),
                            stop=(a == CT - 1),
                        )
                nc.vector.tensor_copy(out=dst[:, mt, :], in_=acc)

            def v_beat(jt):
                """V for all heads of key tile jt, through the psO pool."""
                ps = psO.tile([P, INNER], F32, tag="ov")
                for a in range(CT):
                    nc.tensor.matmul(
                        ps,
                        lhsT=x_b[:, a, jt * P:(jt + 1) * P],
                        rhs=wv_b[:, a, :],
                        start=(a == 0),
                        stop=(a == CT - 1),
                    )
                nc.vector.tensor_copy(
                    out=v_ext[:, jt, :, 0:D],
                    in_=ps.rearrange("p (h d) -> p h d", h=HEADS),
                )

            def out_partial(it):
                """Output projection kk=0..2 partial for row tile it, plus
                bias, parked in SBUF; the tail only needs the kk=3 matmul."""
                pp = psS.tile([P, C], F32, tag="st")
                for kk in range(MT - 1):
                    nc.tensor.matmul(
                        pp,
                        lhsT=oTs[kk][:, it * P:(it + 1) * P],
                        rhs=wo_b[:, kk, :],
                        start=(kk == 0),
                        stop=(kk == MT - 2),
                    )
                nc.vector.tensor_add(part_sb[:, it, :], pp, bo_bc)

            # ---------- attention ----------
            def sim_beat(p, jt, etA, etB):
                stA = psS.tile([P, N], F32, tag="st")
                stB = psS.tile([P, N], F32, tag="st")
                for ib in range(NB):
                    for st, base in ((stA, 0), (stB, D)):
                        nc.tensor.matmul(
                            st[:, ib * 512:(ib + 1) * 512],
                            lhsT=kT[base:base + D, p, jt * P:(jt + 1) * P],
                            rhs=qT[base:base + D, p, ib * 512:(ib + 1) * 512],
                            start=True,
                            stop=True,
                        )
                nc.scalar.activation(
                    out=etA[:, jt, :], in_=stA, func=EXP, bias=zb, scale=SCALE)
                nc.scalar.activation(
                    out=etB[:, jt, :], in_=stB, func=EXP, bias=zb, scale=SCALE)

            def av_beat(p, jt, etA, etB, ovA, ovB):
                for ov, et, h in ((ovA, etA, 2 * p), (ovB, etB, 2 * p + 1)):
                    for ib in range(NB):
                        nc.tensor.matmul(
                            ov[:, ib * 512:(ib + 1) * 512],
                            lhsT=v_ext[:, jt, h, :],
                            rhs=et[:, jt, ib * 512:(ib + 1) * 512],
                            start=(jt == 0),
                            stop=(jt == JT - 1),
                        )

            class Pend:
                pass

            def avtail_and_recip(pend):
                """av jt=7 for the previous pair + PSUM release; denominator
                row spread across 128 partitions via DRAM so the (slow
                per-element) DVE reciprocal runs on 8 elements per lane."""
                av_beat(pend.p, JT - 1, pend.etA, pend.etB, pend.ovA, pend.ovB)
                pend.sb = []
                pend.sds = []
                for ov in (pend.ovA, pend.ovB):
                    ov_sb = ovp.tile([D + 1, N], F32, tag="ovsb")
                    nc.vector.tensor_copy(out=ov_sb, in_=ov)  # frees psO buf
                    sd = dramp.tile([N], F32, tag="sd")
                    nc.sync.dma_start(out=sd, in_=ov_sb[D:D + 1, :])
                    pend.sb.append(ov_sb)
                    pend.sds.append(sd)

            def rep_dma(pend):
                pend.rep = []
                for sd in pend.sds:
                    st2 = small.tile([P, NT], F32, tag="st2")
                    nc.sync.dma_start(
                        out=st2, in_=sd.rearrange("(p k) -> p k", k=NT))
                    rst2 = small.tile([P, NT], F32, tag="rst2")
                    nc.vector.reciprocal(rst2, st2)
                    rsd = dramp.tile([N], F32, tag="rsd")
                    nc.sync.dma_start(
                        out=rsd.rearrange("(p k) -> p k", k=NT), in_=rst2)
                    rep = small.tile([D, N], F32, tag="rep")
                    rsd_ap = rsd[:]
                    nc.sync.dma_start(
                        out=rep,
                        in_=bass.AP(tensor=rsd_ap.tensor, offset=rsd_ap.offset,
                                    ap=[[0, D], [1, N]]),
                    )
                    pend.rep.append(rep)

            def norm_mul(pend):
                for i, base in ((0, 0), (1, D)):
                    nc.vector.tensor_mul(
                        oTs[pend.p][base:base + D, :],
                        pend.sb[i][0:D, :], pend.rep[i])

            # ---------- emission ----------
            proj_beat(0, wk_b, kT)
            proj_beat(0, wq_b, qT)

            pend = None
            for p in range(MT):
                etA = etp.tile([P, JT, N], BF16, tag="et")
                etB = etp.tile([P, JT, N], BF16, tag="et")
                ovA = ovB = None
                for jt in range(JT):
                    sim_beat(p, jt, etA, etB)
                    if p == 0:
                        # v tiles ride psO before ovA/ovB claim it
                        if jt == 0:
                            v_beat(0), v_beat(1), v_beat(2)
                        elif jt == 1:
                            v_beat(3), v_beat(4)
                        elif jt == 2:
                            v_beat(5), v_beat(6), v_beat(7)
                    # last pair: compress the pending chain so oTs[2] exists
                    # before the output-projection partial beats
                    mul_jt = 4 if p == MT - 1 else 5
                    rep_jt = 2 if p == MT - 1 else 3
                    if pend is not None:
                        if jt == 1:
                            avtail_and_recip(pend)
                        elif jt == rep_jt:
                            rep_dma(pend)
                        elif jt == mul_jt:
                            norm_mul(pend)
                            pend = None
                    if jt == 3:
                        ovA = psO.tile([D + 1, N], F32, tag="ov")
                        ovB = psO.tile([D + 1, N], F32, tag="ov")
                    if jt >= 3:
                        av_beat(p, jt - 3, etA, etB, ovA, ovB)
                    # backfilled projection / output-partial beats
                    if p < MT - 1:
                        if jt == 4:
                            proj_beat(p + 1, wk_b, kT)
                        elif jt == 6:
                            proj_beat(p + 1, wq_b, qT)
                    else:
                        if jt >= 5:
                            base_it = 2 * (jt - 5)
                            out_partial(base_it)
                            out_partial(base_it + 1)
                av_beat(p, JT - 3, etA, etB, ovA, ovB)
                av_beat(p, JT - 2, etA, etB, ovA, ovB)
                if p == MT - 1:
                    out_partial(NT - 2)
                    out_partial(NT - 1)
                newp = Pend()
                newp.p, newp.etA, newp.etB, newp.ovA, newp.ovB = \
                    p, etA, etB, ovA, ovB
                pend = newp

            # drain the last pair: norm-chain DVE ops go FIRST on the DVE
            # queue; the last two output partials keep the PE busy meanwhile
            avtail_and_recip(pend)
            rep_dma(pend)
            norm_mul(pend)

            # ---------- output projection tail: kk=3 + partial + store ------
            for it in range(NT):
                pf = psS.tile([P, C], F32, tag="st")
                nc.tensor.matmul(
                    pf,
                    lhsT=oTs[MT - 1][:, it * P:(it + 1) * P],
                    rhs=wo_b[:, MT - 1, :],
                    start=True,
                    stop=True,
                )
                fin = small.tile([P, C], F32, tag="fin")
                nc.vector.tensor_add(fin, pf, part_sb[:, it, :])
                nc.sync.dma_start(out=out_d[it * P:(it + 1) * P, :], in_=fin)

    return nc


BFNP = mybir.dt.np(BF16)


def prepare_in_maps(x, Wq, Wkv, Wo, bo):
    """Host-side prep: reshape x per core, split Wkv, cast matmul inputs
    to bf16 (they feed bf16 matmuls on-chip either way)."""
    x = np.ascontiguousarray(np.asarray(x, np.float32).reshape(B, C, N))
    wkv = np.asarray(Wkv, np.float32)
    wq = np.asarray(Wq, np.float32).astype(BFNP)
    wk = np.ascontiguousarray(wkv[:, :INNER]).astype(BFNP)
    wv = np.ascontiguousarray(wkv[:, INNER:]).astype(BFNP)
    wo = np.asarray(Wo, np.float32).astype(BFNP)
    bo = np.asarray(bo, np.float32)
    return [
        {"x": x[b].astype(BFNP), "Wq": wq, "Wk": wk, "Wv": wv, "Wo": wo,
         "bo": bo}
        for b in range(B)
    ]


def kernel(x, Wq, Wkv, Wo, bo):
    from concourse.bass_utils import run_bass_kernel_spmd

    nc = build_nc()
    nc.compile()
    in_maps = prepare_in_maps(x, Wq, Wkv, Wo, bo)
    res = run_bass_kernel_spmd(nc, in_maps, list(range(B)))
    return np.stack(
        [np.asarray(res.results[b]["out"], dtype=np.float32) for b in range(B)],
        axis=0)


# revision 19
# speedup vs baseline: 1.1757x; 1.0058x over previous
"""Self-contained Trainium2 Bass kernel for nn_Attention (8-head self-attention).

Reference computation (per batch element b):
    xt = x[b].reshape(C, N).T            # (N, C),  N = H*W = 1024
    q  = xt @ Wq                         # (N, 512)
    k, v = split(xt @ Wkv)               # (N, 512) each
    per head h (d=64): sim = q_h k_h^T / 8 ; P = softmax(sim) ; o_h = P v_h
    out[b] = concat_h(o_h) @ Wo + bo     # (N, C)

Sharding: pure data parallel -- core b computes batch element b (8 cores, 8
batch elements, no collectives).

The PE stream is the wall (~378 matmuls x 512 free columns); ScalarE's exp
stream (64 x [128,1024], ~73us) fits underneath it.  Structure:
  - Inputs cast to bf16 on the HOST (half the DMA bytes, no on-chip casts).
    Critical inputs (x / Wk / Wq) interleave per 128-row chunk on the sync
    DMA queue; the mt0 k/q projection accumulates a-outer, chasing them.
  - Head pairs (2p, 2p+1) at partition halves of qT/kT tile p.  Per (pair,
    key-tile) beat: 4 sim MMs -> one [128,1024] exp per head (PSUM->SBUF
    bf16) -> 4 attn@v MMs three beats later.  attn@v keeps a per-head ones
    column (M=65) so softmax denominators cost no extra PE streaming.
  - v-projection tiles ride the psO pool inside pair 0 before the attn@v
    accumulators claim it; kq mt1-3 run as separate k/q beats inside pairs
    (never holding both psS bufs, so the sim/exp pipeline keeps flowing).
  - Norm chain per head: one [65,N] PSUM->SBUF copy (frees PSUM), DVE
    reciprocal_approx_fast on the [1,N] denominator row, DRAM-bounce
    broadcast to [64,N], one DVE multiply.  Deferred one pair to hide
    latency; only the last pair drains serially.
  - Output projection: kk=0..2 partials (+bias) run inside pair 3; the tail
    after the last exp only runs the kk=3 matmuls + final adds + stores.
  - exp LUT table-load (~2.7us) hides behind a warmup exp at t=0.

PSUM (8 banks): tag "st" 2x[128,1024]f32 (sim + projection beats) and tag
"ov" 2x[65->128,1024]f32 (attn@v accumulators / v-proj tiles).
"""

import numpy as np

import concourse.bass as bass
import concourse.mybir as mybir
import concourse.tile as tile
from concourse import bacc

B, C, N = 8, 512, 1024
HEADS, D = 8, 64
INNER = HEADS * D  # 512
SCALE = D ** -0.5
P = 128
CT = C // P       # 4  k-tiles over C
MT = INNER // P   # 4  partition-tiles over inner (one per head pair)
JT = N // P       # 8  key tiles
NT = N // P       # 8  output row tiles
NB = N // 512     # 2  free-dim blocks of 512 over N

F32 = mybir.dt.float32
BF16 = mybir.dt.bfloat16
EXP = mybir.ActivationFunctionType.Exp


def build_nc(debug=False):
    nc = bacc.Bacc(
        "TRN2", target_bir_lowering=False, debug=debug, num_devices=B
    )
    x_d = nc.dram_tensor("x", [C, N], BF16, kind="ExternalInput")
    wq_d = nc.dram_tensor("Wq", [C, INNER], BF16, kind="ExternalInput")
    wk_d = nc.dram_tensor("Wk", [C, INNER], BF16, kind="ExternalInput")
    wv_d = nc.dram_tensor("Wv", [C, INNER], BF16, kind="ExternalInput")
    wo_d = nc.dram_tensor("Wo", [INNER, C], BF16, kind="ExternalInput")
    bo_d = nc.dram_tensor("bo", [C], F32, kind="ExternalInput")
    out_d = nc.dram_tensor("out", [N, C], BF16, kind="ExternalOutput")

    with tile.TileContext(nc) as tc:
        with (
            tc.tile_pool(name="persist", bufs=1) as persist,
            tc.tile_pool(name="etp", bufs=4) as etp,
            tc.tile_pool(name="ovp", bufs=4) as ovp,
            tc.tile_pool(name="small", bufs=3) as small,
            tc.tile_pool(name="dramp", bufs=2, space="DRAM") as dramp,
            tc.tile_pool(name="psS", bufs=2, space="PSUM") as psS,
            tc.tile_pool(name="psO", bufs=2, space="PSUM") as psO,
        ):
            # ---------- consts + exp table warmup (ACT idle at t=0) ----------
            zb = persist.tile([P, 1], F32)
            nc.vector.memset(zb, 0.0)
            warm_i = persist.tile([1, 8], F32)
            nc.vector.memset(warm_i, 0.0)
            warm_o = persist.tile([1, 8], BF16)
            nc.scalar.activation(
                out=warm_o, in_=warm_i, func=EXP, bias=zb[0:1, :], scale=1.0)

            # ---------- input DMAs ----------
            # critical path (x, Wk, Wq chunks) interleaved on the sync queue;
            # Wv / Wo follow; bo broadcast on the gpsimd queue.
            x_b = persist.tile([P, CT, N], BF16)
            wk_b = persist.tile([P, CT, INNER], BF16)
            wq_b = persist.tile([P, CT, INNER], BF16)
            wv_b = persist.tile([P, CT, INNER], BF16)
            x_dv = x_d[:].rearrange("(a p) n -> p a n", p=P)
            wk_dv = wk_d[:].rearrange("(a p) m -> p a m", p=P)
            wq_dv = wq_d[:].rearrange("(a p) m -> p a m", p=P)
            wv_dv = wv_d[:].rearrange("(a p) m -> p a m", p=P)
            for a in range(CT):
                nc.sync.dma_start(out=x_b[:, a, :], in_=x_dv[:, a, :])
                nc.sync.dma_start(out=wk_b[:, a, :], in_=wk_dv[:, a, :])
                nc.sync.dma_start(out=wq_b[:, a, :], in_=wq_dv[:, a, :])
            for a in range(CT):
                nc.sync.dma_start(out=wv_b[:, a, :], in_=wv_dv[:, a, :])
            wo_b = persist.tile([P, MT, C], BF16)
            nc.sync.dma_start(
                out=wo_b, in_=wo_d[:].rearrange("(a p) m -> p a m", p=P))
            bo_bc = persist.tile([P, C], F32)
            bo_ap = bo_d[:]
            nc.gpsimd.dma_start(
                out=bo_bc,
                in_=bass.AP(tensor=bo_ap.tensor, offset=bo_ap.offset,
                            ap=[[0, P], [1, C]]),
            )

            # ---------- persistent attention tensors ----------
            qT = persist.tile([P, MT, N], BF16)
            kT = persist.tile([P, MT, N], BF16)
            v_ext = persist.tile([P, JT, HEADS, D + 1], BF16)
            nc.vector.memset(v_ext[:, :, :, D], 1.0)
            oTs = []
            for m in range(MT):
                oT_m = persist.tile([P, N], BF16, tag=f"oT{m}")
                oTs.append(oT_m)
            part_sb = persist.tile([P, NT, C], F32)

            def proj_beat(mt, w_b, dst):
                """k or q projection for inner tile mt: one psS buf held over
                the a-contraction, then one DVE copy out."""
                acc = psS.tile([P, N], F32, tag="st")
                for a in range(CT):
                    for ib in range(NB):
                        nc.tensor.matmul(
                            acc[:, ib * 512:(ib + 1) * 512],
                            lhsT=w_b[:, a, mt * P:(mt + 1) * P],
                            rhs=x_b[:, a, ib * 512:(ib + 1) * 512],
                            start=(a == 0),
                            stop=(a == CT - 1),
                        )
                nc.vector.tensor_copy(out=dst[:, mt, :], in_=acc)

            def v_beat(jt):
                """V for all heads of key tile jt, through the psO pool."""
                ps = psO.tile([P, INNER], F32, tag="ov")
                for a in range(CT):
                    nc.tensor.matmul(
                        ps,
                        lhsT=x_b[:, a, jt * P:(jt + 1) * P],
                        rhs=wv_b[:, a, :],
                        start=(a == 0),
                        stop=(a == CT - 1),
                    )
                nc.vector.tensor_copy(
                    out=v_ext[:, jt, :, 0:D],
                    in_=ps.rearrange("p (h d) -> p h d", h=HEADS),
                )

            def out_partial(it):
                """Output projection kk=0..2 partial for row tile it, plus
                bias, parked in SBUF; the tail only needs the kk=3 matmul."""
                pp = psS.tile([P, C], F32, tag="st")
                for kk in range(MT - 1):
                    nc.tensor.matmul(
                        pp,
                        lhsT=oTs[kk][:, it * P:(it + 1) * P],
                        rhs=wo_b[:, kk, :],
                        start=(kk == 0),
                        stop=(kk == MT - 2),
                    )
                nc.vector.tensor_add(part_sb[:, it, :], pp, bo_bc)

            # ---------- attention ----------
            def sim_beat(p, jt, etA, etB):
                stA = psS.tile([P, N], F32, tag="st")
                stB = psS.tile([P, N], F32, tag="st")
                for ib in range(NB):
                    for st, base in ((stA, 0), (stB, D)):
                        nc.tensor.matmul(
                            st[:, ib * 512:(ib + 1) * 512],
                            lhsT=kT[base:base + D, p, jt * P:(jt + 1) * P],
                            rhs=qT[base:base + D, p, ib * 512:(ib + 1) * 512],
                            start=True,
                            stop=True,
                        )
                nc.scalar.activation(
                    out=etA[:, jt, :], in_=stA, func=EXP, bias=zb, scale=SCALE)
                nc.scalar.activation(
                    out=etB[:, jt, :], in_=stB, func=EXP, bias=zb, scale=SCALE)

            def av_beat(p, jt, etA, etB, ovA, ovB):
                for ov, et, h in ((ovA, etA, 2 * p), (ovB, etB, 2 * p + 1)):
                    for ib in range(NB):
                        nc.tensor.matmul(
                            ov[:, ib * 512:(ib + 1) * 512],
                            lhsT=v_ext[:, jt, h, :],
                            rhs=et[:, jt, ib * 512:(ib + 1) * 512],
                            start=(jt == 0),
                            stop=(jt == JT - 1),
                        )

            class Pend:
                pass

            def avtail_and_recip(pend):
                """av jt=7 for the previous pair + PSUM release; denominator
                row spread across 128 partitions via DRAM so the (slow
                per-element) DVE reciprocal runs on 8 elements per lane."""
                av_beat(pend.p, JT - 1, pend.etA, pend.etB, pend.ovA, pend.ovB)
                pend.sb = []
                pend.sds = []
                for ov in (pend.ovA, pend.ovB):
                    ov_sb = ovp.tile([D + 1, N], F32, tag="ovsb")
                    nc.vector.tensor_copy(out=ov_sb, in_=ov)  # frees psO buf
                    sd = dramp.tile([N], F32, tag="sd")
                    nc.sync.dma_start(out=sd, in_=ov_sb[D:D + 1, :])
                    pend.sb.append(ov_sb)
                    pend.sds.append(sd)

            def rep_dma(pend):
                pend.rep = []
                for sd in pend.sds:
                    st2 = small.tile([P, NT], F32, tag="st2")
                    nc.sync.dma_start(
                        out=st2, in_=sd.rearrange("(p k) -> p k", k=NT))
                    rst2 = small.tile([P, NT], F32, tag="rst2")
                    nc.vector.reciprocal(rst2, st2)
                    rsd = dramp.tile([N], F32, tag="rsd")
                    nc.sync.dma_start(
                        out=rsd.rearrange("(p k) -> p k", k=NT), in_=rst2)
                    rep = small.tile([D, N], F32, tag="rep")
                    rsd_ap = rsd[:]
                    nc.sync.dma_start(
                        out=rep,
                        in_=bass.AP(tensor=rsd_ap.tensor, offset=rsd_ap.offset,
                                    ap=[[0, D], [1, N]]),
                    )
                    pend.rep.append(rep)

            def norm_mul(pend):
                for i, base in ((0, 0), (1, D)):
                    nc.vector.tensor_mul(
                        oTs[pend.p][base:base + D, :],
                        pend.sb[i][0:D, :], pend.rep[i])

            # ---------- emission ----------
            proj_beat(0, wk_b, kT)
            proj_beat(0, wq_b, qT)

            pend = None
            for p in range(MT):
                etA = etp.tile([P, JT, N], BF16, tag="et")
                etB = etp.tile([P, JT, N], BF16, tag="et")
                ovA = ovB = None
                for jt in range(JT):
                    sim_beat(p, jt, etA, etB)
                    if p == 0:
                        # v tiles ride psO before ovA/ovB claim it
                        if jt == 0:
                            v_beat(0), v_beat(1), v_beat(2)
                        elif jt == 1:
                            v_beat(3), v_beat(4)
                        elif jt == 2:
                            v_beat(5), v_beat(6), v_beat(7)
                    # last pair: compress the pending chain so oTs[2] exists
                    # before the output-projection partial beats
                    mul_jt = 4 if p == MT - 1 else 5
                    rep_jt = 2 if p == MT - 1 else 3
                    if pend is not None:
                        if jt == 1:
                            avtail_and_recip(pend)
                        elif jt == rep_jt:
                            rep_dma(pend)
                        elif jt == mul_jt:
                            norm_mul(pend)
                            pend = None
                    if jt == 3:
                        ovA = psO.tile([D + 1, N], F32, tag="ov")
                        ovB = psO.tile([D + 1, N], F32, tag="ov")
                    if jt >= 3:
                        av_beat(p, jt - 3, etA, etB, ovA, ovB)
                    # backfilled projection / output-partial beats
                    if p < MT - 1:
                        if jt == 4:
                            proj_beat(p + 1, wk_b, kT)
                        elif jt == 6:
                            proj_beat(p + 1, wq_b, qT)
                    else:
                        if jt >= 5:
                            base_it = 2 * (jt - 5)
                            out_partial(base_it)
                            out_partial(base_it + 1)
                av_beat(p, JT - 3, etA, etB, ovA, ovB)
                av_beat(p, JT - 2, etA, etB, ovA, ovB)
                newp = Pend()
                newp.p, newp.etA, newp.etB, newp.ovA, newp.ovB = \
                    p, etA, etB, ovA, ovB
                pend = newp

            # drain the last pair: norm-chain DVE ops go FIRST on the DVE
            # queue; the last two output partials keep the PE busy meanwhile
            avtail_and_recip(pend)
            out_partial(NT - 2)
            out_partial(NT - 1)
            rep_dma(pend)
            norm_mul(pend)

            # ---------- output projection tail: kk=3 + partial + store ------
            for it in range(NT):
                pf = psS.tile([P, C], F32, tag="st")
                nc.tensor.matmul(
                    pf,
                    lhsT=oTs[MT - 1][:, it * P:(it + 1) * P],
                    rhs=wo_b[:, MT - 1, :],
                    start=True,
                    stop=True,
                )
                fin = small.tile([P, C], BF16, tag="fin")
                nc.vector.tensor_add(fin, pf, part_sb[:, it, :])
                eng = nc.sync if it % 2 == 0 else nc.gpsimd
                eng.dma_start(out=out_d[it * P:(it + 1) * P, :], in_=fin)

    return nc


BFNP = mybir.dt.np(BF16)


def prepare_in_maps(x, Wq, Wkv, Wo, bo):
    """Host-side prep: reshape x per core, split Wkv, cast matmul inputs
    to bf16 (they feed bf16 matmuls on-chip either way)."""
    x = np.ascontiguousarray(np.asarray(x, np.float32).reshape(B, C, N))
    wkv = np.asarray(Wkv, np.float32)
    wq = np.asarray(Wq, np.float32).astype(BFNP)
    wk = np.ascontiguousarray(wkv[:, :INNER]).astype(BFNP)
    wv = np.ascontiguousarray(wkv[:, INNER:]).astype(BFNP)
    wo = np.asarray(Wo, np.float32).astype(BFNP)
    bo = np.asarray(bo, np.float32)
    return [
        {"x": x[b].astype(BFNP), "Wq": wq, "Wk": wk, "Wv": wv, "Wo": wo,
         "bo": bo}
        for b in range(B)
    ]


def kernel(x, Wq, Wkv, Wo, bo):
    from concourse.bass_utils import run_bass_kernel_spmd

    nc = build_nc()
    nc.compile()
    in_maps = prepare_in_maps(x, Wq, Wkv, Wo, bo)
    res = run_bass_kernel_spmd(nc, in_maps, list(range(B)))
    return np.stack(
        [np.asarray(res.results[b]["out"], dtype=np.float32) for b in range(B)],
        axis=0)
